# revision 48
# baseline (speedup 1.0000x reference)
"""nn_DTW kernel for 8 Trainium2 NeuronCores (batch data-parallel).

Device computes the cosine-cost matrix (bf16 matmuls) and the full DTW
cumulative table via a skewed column-strip wavefront on the Vector engine;
the host does input layout prep (normalize/transpose/bf16 cast), the
sequential backtrack pointer-chase, and the final logsumexp combine.

Wavefront layout: M=512 columns split into S=16 strips of W=32; partition
p = s*B + b holds strip s of batch b. tc_strip[p, 1+q, 0:33] holds row
(q - R*s) of the DTW table (position 0 is the left-neighbor border column,
positions 1..32 the values). Strips are skewed by R=4 rows so a single
tensor_tensor_scan instruction advances one row of every strip; per row a
DVE pair-min builds m[j] = min(up, diag) and the scan folds in the left
term. The cross-strip dependency flows through PE matmuls that shift each
strip's last column down one strip (+B partitions) into PSUM ring banks
(read as the scan's `initial`) and Activation copies that drop it into
position 0 of the next superstep's slots.
"""

from contextlib import ExitStack

import concourse.bass as bass
import concourse.bacc as bacc
import concourse.tile as tile
from concourse import mybir

F32 = mybir.dt.float32
F16 = mybir.dt.float16
BF16 = mybir.dt.bfloat16
AX = mybir.AxisListType
OP = mybir.AluOpType
ACT = mybir.ActivationFunctionType

BIG = 60000.0   # fp16-safe border sentinel (junk rows add zero cost)

_COMPUTE_OPS = (
    "TensorTensor", "TensorScalarPtr", "TensorReduce", "TensorCopy",
    "Activation", "Matmult", "Memset", "Copy", "TensorScalarAffineSelect",
    "ISA", "Reciprocal", "Iota", "Shift", "MaxIndex", "MatchValueIndex",
)


def _relax_same_engine_sems(nc):
    """Drop semaphore waits that only re-state same-engine program order.

    Each engine executes its queue in order, so a compute instruction never
    needs to wait on its own engine's completion semaphore: every prior
    same-engine instruction has fully executed (and its SBUF writes retired)
    before the next one starts. The tile scheduler still emits those waits;
    removing them eliminates a ~100ns sem-propagation stall between
    back-to-back dependent ops on the same engine. Cross-engine and DMA
    waits are preserved untouched.
    """
    fn = nc.m.functions[0]
    updaters = {}
    for bb in fn.blocks:
        for inst in bb.instructions:
            si = inst.sync_info
            if si is None:
                continue
            is_compute = inst.opcode in _COMPUTE_OPS
            for u in si.on_update:
                if u.sync_type != "semaphore":
                    continue
                tag = (inst.engine, is_compute and u.update_mode == "sem-inc")
                updaters.setdefault(u.ant_name, set()).add(tag)
    own_sem = {}
    for name, tags in updaters.items():
        if len(tags) == 1:
            (eng, ok), = tags
            if ok:
                own_sem.setdefault(eng, set()).add(name)
    ndrop = 0
    prior = {}
    for bb in fn.blocks:
        for inst in bb.instructions:
            si = inst.sync_info
            if si is None:
                continue
            eng = inst.engine
            mine = own_sem.get(eng, ())
            if inst.opcode in _COMPUTE_OPS and si.on_wait:
                keep = []
                for w in si.on_wait:
                    if (w.sync_type == "semaphore"
                            and w.wait_mode == "sem-ge-imm"
                            and w.ant_name in mine):
                        assert w.wait_value <= prior.get((eng, w.ant_name), 0), (
                            f"{inst.name}: wait {w.ant_name}>={w.wait_value} "
                            f"not implied by order "
                            f"({prior.get((eng, w.ant_name), 0)} prior)")
                        ndrop += 1
                    else:
                        keep.append(w)
                if len(keep) != len(si.on_wait):
                    si.on_wait.clear()
                    for w in keep:
                        si.on_wait.append(w)
            for u in si.on_update:
                if (u.sync_type == "semaphore" and u.ant_name in mine
                        and u.update_mode == "sem-inc"):
                    k = (eng, u.ant_name)
                    prior[k] = prior.get(k, 0) + u.update_value
    return ndrop


def _build_cfg(B=8, N=512, M=512, D=256, S=16, W=32, R=4, PART=128):
    assert S * W == M and N % R == 0
    P = S * B
    assert P <= PART
    NT = (N + PART - 1) // PART
    DB = (D + PART - 1) // PART
    PN = min(PART, N)
    PD = min(PART, D)
    NSTEP = N // R
    T_TOT = NSTEP + S - 1
    SLOTS = N + R * S
    SLOT = W + 1
    UPB = PART // R            # supersteps per row-block

    nc = bacc.Bacc("TRN2", target_bir_lowering=False, debug=False)

    xT_in = nc.dram_tensor("xT", [NT, PD, B, DB, PART], BF16,
                           kind="ExternalInput").ap()
    ynT_in = nc.dram_tensor("ynT", [PD, B, DB, M], BF16, kind="ExternalInput").ap()
    xrn_in = nc.dram_tensor("xrn", [PN, B, NT], F32, kind="ExternalInput").ap()
    tc_out = nc.dram_tensor("tc_out", [P, SLOTS, SLOT], F16,
                            kind="ExternalOutput").ap()
    neg_out = nc.dram_tensor("neg_out", [B, 1], F32, kind="ExternalOutput").ap()
    cost_stage = nc.dram_tensor("cost_stage", [NT, B, PN, M], BF16).ap()

    with tile.TileContext(nc) as tcx, ExitStack() as ctx:
        const = ctx.enter_context(tcx.tile_pool(name="const", bufs=1))
        shift8 = const.tile([PART, PART], F16)
        nc.gpsimd.memset(shift8[:], 0.0)
        nc.gpsimd.affine_select(
            out=shift8[:], in_=shift8[:], compare_op=OP.not_equal, fill=1.0,
            base=B, pattern=[[-1, PART]], channel_multiplier=1,
        )
        bigrow = const.tile([1, PART], F16)
        nc.gpsimd.memset(bigrow[:], 0.0)
        nc.gpsimd.memset(bigrow[0:1, 0:B], BIG)
        onesR = const.tile([1, R], F16)
        nc.gpsimd.memset(onesR[:], 1.0)
        oneh = const.tile([PN, B, B], BF16)
        nc.gpsimd.memset(oneh[:], 0.0)
        for b_ in range(B):
            nc.gpsimd.memset(oneh[:, b_, b_:b_ + 1], 1.0)

        bigpad = const.tile([B, 1, SLOT], F16)
        nc.gpsimd.memset(bigpad[:], BIG)
        zcost = const.tile([B, R * S, W], BF16)
        nc.gpsimd.memset(zcost[:], 0.0)

        strip = ctx.enter_context(tcx.tile_pool(name="strip", bufs=1))
        # physical slot 0 = BIG border; logical slot q lives at physical q+1
        tc_strip = strip.tile([P, 1 + SLOTS, SLOT], F16)
        cost_strip = strip.tile([P, SLOTS, W], BF16)
        # BIG borders only where the wavefront reads. Engine ops must start
        # at partition 0, so strip-local inits go through small DMAs instead
        # of memsets.
        nc.gpsimd.memset(tc_strip[:, 0:1, :], BIG)          # dummy slot
        nc.gpsimd.memset(tc_strip[:, :, 0:1], BIG)          # position-0 col

        oper = ctx.enter_context(tcx.tile_pool(name="oper", bufs=1))
        xTall = oper.tile([PD, NT, B, DB, PART], BF16)
        ynTall = oper.tile([PD, B, DB, M], BF16)
        xrn_all = oper.tile([PN, B, NT], F32)
        nc.sync.dma_start(out=xrn_all[:], in_=xrn_in)

        stage = ctx.enter_context(tcx.tile_pool(name="stage", bufs=1))
        cn0_all = stage.tile([PN, B, M], BF16)
        stage_r = ctx.enter_context(tcx.tile_pool(name="stage_r", bufs=3))
        ps_c = ctx.enter_context(tcx.tile_pool(name="ps_c", bufs=3, space="PSUM"))
        ps_neg = ctx.enter_context(tcx.tile_pool(name="ps_neg", bufs=1, space="PSUM"))
        ngb_ps = ps_neg.tile([B, M], F32)
        negsb_pool = ctx.enter_context(tcx.tile_pool(name="negsb", bufs=1))
        negsb = [negsb_pool.tile([B, M], F32, tag=f"negsb{nt}", name=f"negsb{nt}")
                 for nt in range(NT)]
        cn_tiles = {}
        # All pools are created BEFORE any prologue emission: a tile-pool
        # boundary fences every engine queue behind all prior work, and a
        # boundary placed between prologue and wavefront was measured to
        # stall the DVE sequencer ~12us behind the slow tail loads.
        ps_carry = ctx.enter_context(tcx.tile_pool(name="ps_cr", bufs=1, space="PSUM"))
        mpool = ctx.enter_context(tcx.tile_pool(name="mpool", bufs=8))
        carry_tiles = [ps_carry.tile([P, R], F32, tag=f"cr{i}", bufs=1,
                                     name=f"carry{i}")
                       for i in range(4)]
        neg_pool = ctx.enter_context(tcx.tile_pool(name="negp", bufs=1))

        def emit_B_mm0(nt, b):
            rows = min(PART, N - nt * PART)
            psc = ps_c.tile([PN, M], F32, tag="psc", name=f"psc_{nt}_{b}")
            nc.tensor.matmul(
                psc[:rows, :],
                xTall[:PD, nt, b, 0, :rows],
                ynTall[:PD, b, 0, :],
                start=True, stop=(DB == 1),
            )
            return psc

        def emit_B_rest(nt, b, psc):
            rows = min(PART, N - nt * PART)
            for db in range(1, DB):
                nc.tensor.matmul(
                    psc[:rows, :],
                    xTall[:PD, nt, b, db, :rows],
                    ynTall[:PD, b, db, :],
                    start=False, stop=(db == DB - 1),
                )
            if nt == 0:
                cn = cn0_all[:, b, :]
            else:
                cnt = stage_r.tile([PN, M], BF16, tag="cn", name=f"cn_{nt}_{b}")
                cn_tiles[(nt, b)] = cnt
                cn = cnt[:, :]
            nc.scalar.activation(cn[:rows], psc[:rows], ACT.Copy,
                                 scale=xrn_all[:rows, b, nt:nt + 1], bias=1.0)
            eng = nc.scalar if b % 2 == 0 else nc.sync
            eng.dma_start(out=cost_stage[nt, b], in_=cn[:rows])

        def emit_ngb(nt, b):
            rows = min(PART, N - nt * PART)
            if nt == 0:
                cn = cn0_all[:, b, :]
            else:
                cn = cn_tiles.pop((nt, b))[:, :]
            nc.tensor.matmul(
                ngb_ps[:, :],
                oneh[:rows, b, :],
                cn[:rows],
                start=(b == 0), stop=(b == B - 1),
                skip_group_check=True,
            )
            if b == B - 1:
                nc.scalar.copy(negsb[nt][:, :], ngb_ps[:, :])

        _Q = [nc.sync, nc.scalar]

        def emit_hop2(nt, s0, s1):
            rows = min(PART, N - nt * PART)
            for s in range(s0, s1):
                src = cost_stage[nt, :, :, s * W:(s + 1) * W]
                eng = _Q[s % 2]
                eng.dma_start(
                    out=cost_strip[s * B:s * B + B,
                                   R * s + nt * PART:R * s + nt * PART + rows,
                                   0:W],
                    in_=src)

        def emit_bigpad(s):
            # strip s's row -1 (logical slot R*s-1 -> physical R*s)
            nc.gpsimd.dma_start(
                out=tc_strip[s * B:(s + 1) * B, R * s:R * s + 1, :],
                in_=bigpad[:, :, :])

        def emit_zcost(s):
            # finished strips keep scanning past their last row; zero cost
            # there keeps that junk finite (the carry matmul reads it).
            npad = R * (S - s)
            _Q[s % 2].dma_start(
                out=cost_strip[s * B:(s + 1) * B, R * s + N:SLOTS, :],
                in_=zcost[:, 0:npad, :])

        # ---------------- prologue: loads + block-0 cost ----------------
        # Loads, cost staging, and hop2 ride the two hardware-DGE queues
        # (SP/Act); the strip border pads go through the gpsimd software-DGE
        # queue, which executes serially on the idle Pool engine and has far
        # deadlines (pad s is first read at superstep s).
        def emit_B0_half(b, mh):
            mc = slice(mh * (M // 2), (mh + 1) * (M // 2))
            psc = ps_c.tile([PN, M // 2], F32, tag="psc", name=f"psch_{b}_{mh}")
            for db in range(DB):
                nc.tensor.matmul(
                    psc[:, :],
                    xTall[:PD, 0, b, db, :],
                    ynTall[:PD, b, db, mc],
                    start=(db == 0), stop=(db == DB - 1),
                )
            nc.scalar.activation(cn0_all[:, b, mc], psc[:, :], ACT.Copy,
                                 scale=xrn_all[:, b, 0:1], bias=1.0)

        # Each dma_start costs its queue's sequencer ~1.25us and loads are
        # bandwidth-serialized, so block 0 is computed in m-halves: the first
        # half (and hop2 strips 0..7) only needs xT rows 0..127 plus half of
        # ynT. Strip pads ride the gpsimd software-DGE queue (idle Pool
        # engine, far deadlines: pad s is first read at superstep s).
        MH = M // 2
        with tcx.high_priority():
            nc.sync.dma_start(out=xTall[:, 0], in_=xT_in[0])
            nc.scalar.dma_start(out=ynTall[:, :, 0, 0:MH],
                                in_=ynT_in[:, :, 0, 0:MH])
            nc.sync.dma_start(out=ynTall[:, :, 1, 0:MH],
                              in_=ynT_in[:, :, 1, 0:MH])
        for s in range(1, 9):
            emit_bigpad(s)
        # Tail loads ride the Pool software-DGE queue behind the first pads:
        # they must not generate descriptors before the urgent cn/hop2
        # transfers or the FIFO DMA pipe delays the wavefront start.
        nc.gpsimd.dma_start(out=ynTall[:, :, 0, MH:M], in_=ynT_in[:, :, 0, MH:M])
        nc.gpsimd.dma_start(out=ynTall[:, :, 1, MH:M], in_=ynT_in[:, :, 1, MH:M])
        for nt_ in range(1, NT):
            nc.gpsimd.dma_start(out=xTall[:, nt_], in_=xT_in[nt_])
        for s in range(9, S):
            emit_bigpad(s)
        for b in range(B):
            emit_B0_half(b, 0)
        # one staged cost write per m-half (single trigger, ~0.7us transfer)
        nc.sync.dma_start(
            out=cost_stage[0, :, :, 0:MH].rearrange("b p m -> p b m"),
            in_=cn0_all[:, :, 0:MH])
        emit_hop2(0, 0, 8)
        for b in range(B):
            emit_B0_half(b, 1)
        nc.scalar.dma_start(
            out=cost_stage[0, :, :, MH:M].rearrange("b p m -> p b m"),
            in_=cn0_all[:, :, MH:M])
        emit_hop2(0, 8, S)
        pscs = {}

        # ---------------- DTW wavefront ----------------

        def emit_carry(U, c0, c1):
            """cps[p, c] = left strip's last column at slot base+c, shifted
            one strip down; BIG for strip 0. The Act copy drops it into
            position 0 of superstep U+1's slots (the m-input left border)."""
            base = R * U
            bnd = min(S - 1, U + 1) * B
            # Separate PSUM banks for the two per-superstep chunks: PSUM
            # deps are bank-granular, so chunk B's matmul must not share a
            # bank with chunk A's (already copy-read) tile.
            ci = (U + 1) % 2 + (2 if c0 >= 2 else 0)
            cps = carry_tiles[ci]
            nc.tensor.matmul(
                cps[0:bnd + B, c0:c1],
                shift8[0:bnd, 0:bnd + B],
                tc_strip[0:bnd, base + c0 + 1:base + c1 + 1, SLOT - 1:SLOT],
                start=True, stop=False, skip_group_check=True,
            )
            nc.tensor.matmul(
                cps[0:bnd + B, c0:c1],
                bigrow[0:1, 0:bnd + B],
                onesR[0:1, 0:c1 - c0],
                start=False, stop=True, skip_group_check=True,
            )
            nc.scalar.copy(
                tc_strip[0:bnd + B, base + R + c0 + 1:base + R + c1 + 1, 0:1],
                cps[0:bnd + B, c0:c1])
            return ci

        state = {"prev_carry": None, "out_lo": 0}

        def emit_stageC(U0, U1, sprinkle=None):
            for U in range(U0, U1):
                smax = min(S - 1, U)
                phi = (smax + 1) * B
                base = R * U
                for k in range(R):
                    q = base + k
                    mt_ = mpool.tile([P, W], F16, tag="m", name=f"m_{U}_{k}")
                    nc.vector.tensor_tensor(
                        mt_[0:phi, :],
                        tc_strip[0:phi, q, 0:W],
                        tc_strip[0:phi, q, 1:SLOT],
                        OP.min,
                    )
                    if U == 0:
                        init = 0.0 if k == 0 else BIG
                    else:
                        ci = U % 2 + (2 if k >= 2 else 0)
                        init = carry_tiles[ci][0:phi, k:k + 1]
                    nc.vector.tensor_tensor_scan(
                        tc_strip[0:phi, q + 1, 1:SLOT],
                        mt_[0:phi, :],
                        cost_strip[0:phi, q, :],
                        init,
                        OP.min,
                        OP.add,
                    )
                    if U + 1 < T_TOT:
                        if k == 1:
                            emit_carry(U, 0, 2)
                        elif k == R - 1:
                            emit_carry(U, 2, R)
                    if sprinkle:
                        for u_off, k_off, fn_ in sprinkle:
                            if U == U0 + u_off and k == k_off:
                                fn_()
                # stream finished slots out every 8 supersteps (4 near the
                # end, to shrink the final tail dump)
                if ((U + 1) % 8 == 0 or ((U + 1) % 4 == 0 and U >= 128)) \
                        and U + 1 < T_TOT:
                    lo_d, hi_d = state["out_lo"], (U + 1) * R
                    nc.sync.dma_start(out=tc_out[:, lo_d:hi_d, :],
                                      in_=tc_strip[:, lo_d + 1:hi_d + 1, :])
                    state["out_lo"] = hi_d

        # Sprinkle later blocks' cost work into the wavefront. Block nt is
        # first needed at superstep nt*UPB; emit it across the preceding
        # chunk, one PE matmul per superstep to keep the carry matmuls from
        # queueing behind it.
        def mk(fn_, *a):
            return lambda: fn_(*a)

        def chunk_sprinkle(nt):
            spr = []
            u0 = 6 if nt == 1 else 0   # chunk 1 waits for the xT tail load
            for b in range(B):
                u = u0 + 2 * b
                spr.append((u, 0, mk(lambda n_, b_: pscs.__setitem__(
                    b_, emit_B_mm0(n_, b_)), nt, b)))
                spr.append((u + 1, 0, mk(lambda n_, b_: emit_B_rest(
                    n_, b_, pscs.pop(b_)), nt, b)))
                spr.append((u + 2, 0, mk(emit_ngb, nt, b)))
            if nt == 1:
                for s in range(S):
                    spr.append((8 + s, 2, mk(emit_zcost, s)))
            spr.append((UPB - 3, 3, mk(emit_hop2, nt, 0, 8)))
            spr.append((UPB - 2, 3, mk(emit_hop2, nt, 8, S)))
            return spr

        def emit_neg():
            # neg = logsumexp over m, entirely off the DVE chain: reductions
            # on the idle Pool engine, transcendentals on Act.
            negsum = neg_pool.tile([B, M], F32)
            nc.gpsimd.tensor_tensor(negsum[:], negsb[0][:, :], negsb[1][:, :],
                                    OP.add)
            for nt in range(2, NT):
                nc.gpsimd.tensor_tensor(negsum[:], negsum[:], negsb[nt][:, :],
                                        OP.add)
            # Constant logsumexp shift: column sums concentrate at N +- ~10
            # (cost = 1 - cos, cos ~ N(0, 1/D)), so exp(colsum - N) is safe.
            sh = neg_pool.tile([B, M], F32)
            nc.gpsimd.tensor_scalar_add(sh[:], negsum[:], -float(N))
            ex = neg_pool.tile([B, M], F32)
            esum = neg_pool.tile([B, 1], F32)
            nc.scalar.activation(ex[:], sh[:], ACT.Exp, accum_out=esum[:])
            lg = neg_pool.tile([B, 1], F32)
            nc.scalar.activation(lg[:], esum[:], ACT.Ln)
            negv = neg_pool.tile([B, 1], F32)
            nc.gpsimd.tensor_scalar_add(negv[:], lg[:], float(N))
            nc.sync.dma_start(out=neg_out[:, :], in_=negv[:])

        for nt in range(1, NT):
            emit_stageC((nt - 1) * UPB, nt * UPB, sprinkle=chunk_sprinkle(nt))
        # block 0's ngb group runs last (cn0_all is persistent, and PSUM
        # accumulation groups on the shared bank must not interleave)
        spr_last = [(2 + 2 * b, 0, mk(emit_ngb, 0, b)) for b in range(B)]
        spr_last.append((2 + 2 * B + 2, 1, emit_neg))
        emit_stageC((NT - 1) * UPB, T_TOT, sprinkle=spr_last)

        lo_d = state["out_lo"]
        nc.sync.dma_start(out=tc_out[:, lo_d:SLOTS, :],
                          in_=tc_strip[:, lo_d + 1:SLOTS + 1, :])

    # NOTE: dropping same-engine RAW semaphores is NOT sound here — the DVE
    # pipelines back-to-back instructions, so a consumer issued right behind
    # its producer reads partially-written SBUF (verified empirically).
    nc.compile()
    return nc


# ---------------------------------------------------------------------------
# Host-side driver: sharding, layout prep, run, unskew, backtrack, final loss
# ---------------------------------------------------------------------------
import numpy as np
import ml_dtypes

BF16_NP = ml_dtypes.bfloat16
EPS = 1e-8

B_TOT, N_G, M_G, D_G = 64, 512, 512, 256
N_CORES = 8
B_LOC = B_TOT // N_CORES
S_G, W_G, R_G = 16, 32, 4
P_G = S_G * B_LOC
SLOTS_G = N_G + R_G * S_G
SLOT_G = W_G + 1
NT_G = N_G // 128
DB_G = D_G // 128

_NC_CACHE = {}


def _get_nc():
    if "nc" not in _NC_CACHE:
        _NC_CACHE["nc"] = _build_cfg(B=B_LOC, N=N_G, M=M_G, D=D_G,
                                     S=S_G, W=W_G, R=R_G)
    return _NC_CACHE["nc"]


def _unskew(tc_skew):
    tc = np.empty((B_LOC, N_G, M_G), np.float32)
    for s in range(S_G):
        for b in range(B_LOC):
            tc[b, :, s * W_G:(s + 1) * W_G] = \
                tc_skew[s * B_LOC + b, R_G * s:R_G * s + N_G, 1:SLOT_G]
    return tc


def _prep_core(x, y, ny_inv, nx_inv):
    """Device input layout for one core's batch slice.

    xT/ynT: [128, B_LOC, DB, N] bf16 with element [p,b,db,n] = t[b, n, db*128+p]
    xrn:    [128, B_LOC, NT] f32 = -1/||x_row|| arranged row-block-major.
    """
    yn = y * ny_inv[..., None]
    xt = np.ascontiguousarray(x.transpose(2, 0, 1))   # [D, B, N]
    ynt = np.ascontiguousarray(yn.transpose(2, 0, 1))
    # xT block-major: [NT, 128, B, DB, 128], element = x[b, nt*128+j, db*128+p]
    xT = np.ascontiguousarray(
        xt.reshape(DB_G, 128, B_LOC, NT_G, 128).transpose(3, 1, 2, 0, 4)
    ).astype(BF16_NP)
    ynT = np.ascontiguousarray(
        ynt.reshape(DB_G, 128, B_LOC, M_G).transpose(1, 2, 0, 3)).astype(BF16_NP)
    xrn = np.ascontiguousarray(
        (-nx_inv).reshape(B_LOC, NT_G, 128).transpose(2, 0, 1)).astype(np.float32)
    return {"xT": xT, "ynT": ynT, "xrn": xrn}


def _host_finish(tc, x, y, neg):
    """Backtrack walk on the device tc + pos logsumexp (host side)."""
    Bt, Nn, Mm = tc.shape
    xn = x / np.maximum(np.linalg.norm(x, axis=-1, keepdims=True), EPS)
    yn = y / np.maximum(np.linalg.norm(y, axis=-1, keepdims=True), EPS)
    bidx = np.arange(Bt)
    i = np.full(Bt, Nn - 1, np.int64)
    j = np.full(Bt, Mm - 1, np.int64)
    Is, Js, Vs = [i.copy()], [j.copy()], [np.ones(Bt, bool)]
    active = (i > 0) & (j > 0)
    while active.any():
        a = tc[bidx, np.maximum(i - 1, 0), np.maximum(j - 1, 0)]
        bb = tc[bidx, np.maximum(i - 1, 0), j]
        c = tc[bidx, i, np.maximum(j - 1, 0)]
        diag = (a <= bb) & (a <= c)
        up = (~diag) & (bb <= c)
        ni = np.where(diag | up, i - 1, i)
        nj = np.where(diag | (~up), j - 1, j)
        i = np.where(active, ni, i)
        j = np.where(active, nj, j)
        Is.append(i.copy())
        Js.append(j.copy())
        Vs.append(active.copy())
        active = (i > 0) & (j > 0)
    at00 = (i == 0) & (j == 0)
    Is.append(np.zeros(Bt, np.int64))
    Js.append(np.zeros(Bt, np.int64))
    Vs.append(~at00)

    IS = np.stack(Is, 1)
    JS = np.stack(Js, 1)
    VS = np.stack(Vs, 1)
    costs = 1.0 - np.einsum("bld,bld->bl",
                            xn[bidx[:, None], IS], yn[bidx[:, None], JS])
    colsum = np.zeros((Bt, Mm), np.float32)
    np.add.at(colsum, (bidx[:, None], JS),
              np.where(VS, costs, 0.0).astype(np.float32))
    mxv = colsum.max(axis=1, keepdims=True)
    pos = (mxv + np.log(np.sum(np.exp(colsum - mxv),
                               axis=1, keepdims=True))).squeeze(1)
    return (pos.astype(np.float32) - neg).astype(np.float32)


def run_device(x, y, **kw):
    from concourse import bass_utils

    nc = _get_nc()
    nx_inv = 1.0 / np.maximum(np.linalg.norm(x, axis=-1), EPS)  # [B, N]
    ny_inv = 1.0 / np.maximum(np.linalg.norm(y, axis=-1), EPS)  # [B, M]
    in_maps = []
    for c in range(N_CORES):
        sl = slice(c * B_LOC, (c + 1) * B_LOC)
        in_maps.append(_prep_core(x[sl], y[sl], ny_inv[sl], nx_inv[sl]))
    res = bass_utils.run_bass_kernel_spmd(nc, in_maps, list(range(N_CORES)), **kw)
    tc = np.empty((B_TOT, N_G, M_G), np.float32)
    neg = np.empty(B_TOT, np.float32)
    for c in range(N_CORES):
        out = res.results[c]
        tc[c * B_LOC:(c + 1) * B_LOC] = _unskew(out["tc_out"])
        neg[c * B_LOC:(c + 1) * B_LOC] = out["neg_out"].reshape(B_LOC)
    return tc, neg, res


def kernel(x, y):
    x = np.asarray(x, dtype=np.float32)
    y = np.asarray(y, dtype=np.float32)
    tc, neg, _ = run_device(x, y)
    return _host_finish(tc, x, y, neg)
